# revision 2
# baseline (speedup 1.0000x reference)
"""Trainium2 Bass kernel for nn_HDCNN (4-layer hyperbolic dilated CNN), v2.

Data-parallel over 8 NeuronCores (4096 rows each). Feature-transposed layout:
activations live as [feature, batch] bf16 tiles; the 64-tap full convolution is
2 dense bf16 128x128 matmuls per output 128-chunk (W_lo/W_hi band matrices).

Math (same reduction as v1, validated against the reference): the Poincare
projection always triggers, so each layer is
    h' = relu(u + delta * y),  u = conv(h), delta = (CY/M) * n / cx,
    n = ||u||, cx = 1 + y2 + 2*M*<u,y>/n,
with the final layer scaled by m = M*cx/(n*den). Per-sample stats (s = n^2 via
squares, d = <u,y> via the precomputed cross-correlation beta on h) are reduced
with column-packed M=2 matmuls on 4 concurrent PE column strips.

Precision: bf16 inputs/weights/activations with fp32 PSUM accumulation.
Host-side numpy simulation of this exact pipeline measures l2 ~ 2.6e-3 vs the
fp32 reference (tolerance 2e-2).
"""
import os
import sys

for _p in ("/opt/trn_rl_repo", "/root/.axon_site/_ro/trn_rl_repo"):
    if _p not in sys.path and os.path.isdir(_p):
        sys.path.append(_p)

import numpy as np
import ml_dtypes
import concourse.bacc as bacc
import concourse.mybir as mybir
import concourse.tile as tile
from concourse import bass_utils

F32 = mybir.dt.float32
BF = mybir.dt.bfloat16
OP = mybir.AluOpType
AX = mybir.AxisListType
BF_NP = ml_dtypes.bfloat16

NCORES = 8
BATCH = 32768
INSIZE = 1024
FLEN = 64
NUM_LAYERS = 4
ROWS_PER_CORE = BATCH // NCORES          # 4096
NB = 512                                  # batch columns per tile
NTILES = ROWS_PER_CORE // NB              # 8
MAXNORM = 1.0 - 4e-3
COEF_Y = 1.0 - MAXNORM * MAXNORM

LIN = [INSIZE + FLEN * i for i in range(NUM_LAYERS)]          # 1024 1088 1152 1216
LOUT = [l + FLEN for l in LIN]                                # 1088 1152 1216 1280
NIN_C = [(l + 127) // 128 for l in LIN]                       # 8 9 9 10
NOUT_C = [(l + 127) // 128 for l in LOUT]                     # 9 9 10 10
NINMAX = max(NIN_C)
NOUTMAX = max(NOUT_C)

# Fraction denominator for relu placement: c % RELU_SPLIT == 0 goes to DVE,
# the rest to GPSIMD.
RELU_SPLIT = int(os.environ.get("HDCNN_RELU_SPLIT", "3"))


def host_prep(w, b_list):
    """Replicated parameter layouts (band matrices + correlations), bf16."""
    prep = {}
    wlo = np.zeros((NUM_LAYERS, 128, 128), np.float32)
    whi = np.zeros((NUM_LAYERS, 128, 128), np.float32)
    k = np.arange(128)[:, None]
    m = np.arange(128)[None, :]
    for i in range(NUM_LAYERS):
        dif = m - k
        sel = (dif >= 0) & (dif < FLEN)
        whi[i][sel] = w[i][dif[sel]]
        dif2 = m + 128 - k
        sel2 = (dif2 >= 0) & (dif2 < FLEN)
        wlo[i][sel2] = w[i][dif2[sel2]]
    prep["wlo"] = wlo.astype(BF_NP)
    prep["whi"] = whi.astype(BF_NP)

    sones2 = np.zeros((128, 2), np.float32)
    sones2[:, 0] = 1.0
    prep["sones2"] = sones2.astype(BF_NP)

    beta2 = np.zeros((NUM_LAYERS, 128, NINMAX, 2), np.float32)
    ycolp = np.zeros((NUM_LAYERS, 128, NOUTMAX), np.float32)
    y2c = np.zeros((128, NUM_LAYERS, 2), np.float32)
    for i in range(NUM_LAYERS):
        b64 = b_list[i].astype(np.float64)
        bt = np.correlate(b64, w[i].astype(np.float64), mode="valid")[: LIN[i]]
        bpad = np.zeros(NIN_C[i] * 128)
        bpad[: LIN[i]] = bt
        beta2[i, :, : NIN_C[i], 1] = bpad.reshape(NIN_C[i], 128).T
        ypad = np.zeros(NOUT_C[i] * 128)
        ypad[: LOUT[i]] = b64 * (COEF_Y / MAXNORM)
        ycolp[i, :, : NOUT_C[i]] = ypad.reshape(NOUT_C[i], 128).T
        y2 = np.float32(np.sum(b_list[i].astype(np.float32) ** 2, dtype=np.float32))
        y2c[:, i, 0] = np.float32(1.0) + y2                                # cx const
        y2c[:, i, 1] = np.float32(1.0) + np.float32(MAXNORM * MAXNORM) * y2  # den const
    prep["beta2"] = beta2.astype(BF_NP)
    prep["ycolp"] = ycolp.astype(BF_NP)
    prep["y2c"] = y2c

    # Broadcast selectors: ebc[b] is lhsT [8, 128]; row b set to 1 (b<4, delta
    # blocks) or MAXNORM (b>=4, m blocks), everything else 0.
    ebc = np.zeros((8, 8, 128), np.float32)
    for b in range(8):
        ebc[b, b, :] = 1.0 if b < 4 else MAXNORM
    prep["ebc"] = ebc.astype(BF_NP)

    prep["idbf"] = np.eye(128, dtype=BF_NP)
    prep["idf"] = np.eye(128, dtype=np.float32)
    return prep


def build_program(ntiles=NTILES, reps=1):
    nc = bacc.Bacc("TRN2", target_bir_lowering=False, debug=False)
    ncols = ntiles * NB

    hkr = nc.dram_tensor("hkr", [ncols, INSIZE], BF, kind="ExternalInput")
    d_wlo = nc.dram_tensor("wlo", [NUM_LAYERS, 128, 128], BF, kind="ExternalInput")
    d_whi = nc.dram_tensor("whi", [NUM_LAYERS, 128, 128], BF, kind="ExternalInput")
    d_sones2 = nc.dram_tensor("sones2", [128, 2], BF, kind="ExternalInput")
    d_beta2 = nc.dram_tensor("beta2", [NUM_LAYERS, 128, NINMAX, 2], BF, kind="ExternalInput")
    d_ycolp = nc.dram_tensor("ycolp", [NUM_LAYERS, 128, NOUTMAX], BF, kind="ExternalInput")
    d_y2c = nc.dram_tensor("y2c", [128, NUM_LAYERS, 2], F32, kind="ExternalInput")
    d_ebc = nc.dram_tensor("ebc", [8, 8, 128], BF, kind="ExternalInput")
    d_idbf = nc.dram_tensor("idbf", [128, 128], BF, kind="ExternalInput")
    d_idf = nc.dram_tensor("idf", [128, 128], F32, kind="ExternalInput")
    outT = nc.dram_tensor("outT", [LOUT[-1], ncols], BF, kind="ExternalOutput")
    out_v = outT.rearrange("(c p) n -> p c n", p=128)

    with tile.TileContext(nc) as tc:
        with (
            tc.tile_pool(name="singles", bufs=1) as singles,
            tc.tile_pool(name="hkp", bufs=2) as hkp,
            tc.tile_pool(name="usbp", bufs=2) as usbp,
            tc.tile_pool(name="hp", bufs=2) as hp,
            tc.tile_pool(name="sqp", bufs=4) as sqp,
            tc.tile_pool(name="sdp", bufs=2) as sdp,
            tc.tile_pool(name="scp", bufs=2) as scp,
            tc.tile_pool(name="rowp", bufs=2) as rowp,
            tc.tile_pool(name="dbp", bufs=2) as dbp,
            tc.tile_pool(name="outp", bufs=4) as outp,
            tc.tile_pool(name="cvps", bufs=3, space="PSUM") as cvps,
            tc.tile_pool(name="stps", bufs=1, space="PSUM") as stps,
            tc.tile_pool(name="rwps", bufs=2, space="PSUM") as rwps,
            tc.tile_pool(name="bcps", bufs=1, space="PSUM") as bcps,
        ):
            s_wlo = singles.tile([128, NUM_LAYERS, 128], BF, tag="wlo")
            s_whi = singles.tile([128, NUM_LAYERS, 128], BF, tag="whi")
            s_sones2 = singles.tile([128, 2], BF, tag="sones2")
            s_beta2 = singles.tile([128, NUM_LAYERS, NINMAX, 2], BF, tag="beta2")
            s_ycolp = singles.tile([128, NUM_LAYERS, NOUTMAX], BF, tag="ycolp")
            s_y2c = singles.tile([128, NUM_LAYERS, 2], F32, tag="y2c")
            s_ebc = singles.tile([8, 8, 128], BF, tag="ebc")
            s_idbf = singles.tile([128, 128], BF, tag="idbf")
            s_idf = singles.tile([128, 128], F32, tag="idf")
            nc.sync.dma_start(out=s_wlo, in_=d_wlo.rearrange("l p m -> p l m"))
            nc.sync.dma_start(out=s_whi, in_=d_whi.rearrange("l p m -> p l m"))
            nc.sync.dma_start(out=s_sones2, in_=d_sones2[:])
            nc.sync.dma_start(out=s_beta2, in_=d_beta2.rearrange("l p c t -> p l c t"))
            nc.sync.dma_start(out=s_ycolp, in_=d_ycolp.rearrange("l p c -> p l c"))
            nc.sync.dma_start(out=s_y2c, in_=d_y2c[:])
            nc.sync.dma_start(out=s_ebc, in_=d_ebc.rearrange("b r m -> r b m"))
            nc.sync.dma_start(out=s_idbf, in_=d_idbf[:])
            nc.sync.dma_start(out=s_idf, in_=d_idf[:])

            for jj in range(ntiles * reps):
                j = jj % ntiles
                ncol = slice(j * NB, (j + 1) * NB)

                hkt = hkp.tile([128, NIN_C[0], NB], BF, tag="hk")
                for c in range(NIN_C[0]):
                    nc.sync.dma_start_transpose(
                        hkt[:, c, :], hkr[j * NB : (j + 1) * NB, 128 * c : 128 * (c + 1)]
                    )
                u = hkt

                for li in range(NUM_LAYERS):
                    nin_c, nout_c = NIN_C[li], NOUT_C[li]
                    last = li == NUM_LAYERS - 1
                    strip_last_s = {
                        sj: max(c for c in range(nout_c) if c % 4 == sj)
                        for sj in range(4)
                    }

                    # --- stats tile; d-part first (only needs layer input) ---
                    ST = stps.tile([128, NB], F32, tag="st")
                    for c in range(nin_c):
                        sj = c % 4
                        nc.tensor.matmul(
                            ST[32 * sj : 32 * sj + 2, :],
                            s_beta2[:, li, c, :],
                            u[:, c, :],
                            start=(c < 4),
                            stop=False,
                            tile_position=(0, 32 * sj),
                        )

                    # --- conv chunks: 2 bf16 matmuls each; evac + square ---
                    usb = usbp.tile([128, nout_c, NB], BF, tag=f"u{li}")
                    for c in range(nout_c):
                        pcv = cvps.tile([128, NB], F32, tag="cv")
                        ops = []
                        if 1 <= c <= nin_c:
                            ops.append((s_wlo[:, li, :], c - 1))
                        if c < nin_c:
                            ops.append((s_whi[:, li, :], c))
                        for oi, (lhs, uc) in enumerate(ops):
                            nc.tensor.matmul(
                                pcv, lhs, u[:, uc, :],
                                start=(oi == 0), stop=(oi == len(ops) - 1),
                            )
                        nc.scalar.copy(usb[:, c, :], pcv)
                        sq = sqp.tile([128, NB], BF, tag="sq")
                        nc.vector.tensor_tensor(sq, usb[:, c, :], usb[:, c, :], OP.mult)
                        sj = c % 4
                        nc.tensor.matmul(
                            ST[32 * sj : 32 * sj + 2, :],
                            s_sones2,
                            sq,
                            start=False,
                            stop=(c == strip_last_s[sj]),
                            tile_position=(0, 32 * sj),
                        )

                    # --- stats -> per-sample scalars ([128, 4] layout) ---
                    sd = sdp.tile([128, NB], F32, tag="sd")
                    nc.scalar.copy(sd, ST)
                    tp4 = stps.tile([128, 4, 128], F32, tag="tp4")
                    for b in range(4):
                        nc.tensor.transpose(
                            tp4[:, b, :], sd[:, 128 * b : 128 * (b + 1)], s_idf
                        )
                    sc = scp.tile([128, 28], F32, tag="sc")
                    S4, D4 = sc[:, 0:4], sc[:, 4:8]
                    nS, rn = sc[:, 8:12], sc[:, 12:16]
                    t0, cx, rcx = sc[:, 16:20], sc[:, 20:24], sc[:, 24:28]
                    nc.vector.tensor_reduce(S4, tp4[:, :, 0:128:32], AX.X, OP.add)
                    nc.vector.tensor_reduce(D4, tp4[:, :, 1:128:32], AX.X, OP.add)
                    nc.scalar.sqrt(nS, S4)
                    nc.vector.reciprocal(rn, nS)
                    nc.vector.tensor_tensor(t0, D4, rn, OP.mult)
                    nc.vector.tensor_scalar(
                        cx, t0, 2.0 * MAXNORM, s_y2c[:, li, 0:1], OP.mult, OP.add
                    )
                    nc.vector.reciprocal(rcx, cx)
                    dm = scp.tile([128, 8], F32, tag="dm")
                    nc.vector.tensor_tensor(dm[:, 0:4], nS, rcx, OP.mult)  # delta
                    if last:
                        sc2 = scp.tile([128, 12], F32, tag="sc2")
                        den, rden, m1 = sc2[:, 0:4], sc2[:, 4:8], sc2[:, 8:12]
                        nc.vector.tensor_scalar(
                            den, t0, 2.0 * MAXNORM, s_y2c[:, li, 1:2], OP.mult, OP.add
                        )
                        nc.vector.reciprocal(rden, den)
                        nc.vector.tensor_tensor(m1, cx, rn, OP.mult)
                        nc.vector.tensor_tensor(dm[:, 4:8], m1, rden, OP.mult)  # m/M

                    # --- broadcast delta (and m) to [128, NB] bf16 tiles ---
                    nrow = 8 if last else 4
                    rw = rwps.tile([8, 128], F32, tag="rw")
                    nc.tensor.transpose(rw[0:nrow, :], dm[:, 0:nrow], s_idf)
                    rws = rowp.tile([8, 128], BF, tag="rws")
                    nc.scalar.copy(rws[0:nrow, :], rw[0:nrow, :])
                    dbc = bcps.tile([128, NB], F32, tag="bc")
                    for b in range(4):
                        nc.tensor.matmul(
                            dbc[:, 128 * b : 128 * (b + 1)],
                            s_ebc[:, b, :],
                            rws[:, :],
                            start=True, stop=True, tile_position=(0, 0),
                        )
                    dbs = dbp.tile([128, NB], BF, tag="dbs")
                    nc.scalar.copy(dbs, dbc)
                    if last:
                        mbc = bcps.tile([128, NB], F32, tag="bc")
                        for b in range(4):
                            nc.tensor.matmul(
                                mbc[:, 128 * b : 128 * (b + 1)],
                                s_ebc[:, 4 + b, :],
                                rws[:, :],
                                start=True, stop=True, tile_position=(0, 0),
                            )
                        mbs = dbp.tile([128, NB], BF, tag="mbs")
                        nc.scalar.copy(mbs, mbc)

                    # --- q update (in place), relu, output ---
                    if not last:
                        hn = hp.tile([128, nout_c, NB], BF, tag=f"h{li + 1}")
                    for c in range(nout_c):
                        nc.vector.scalar_tensor_tensor(
                            usb[:, c, :], dbs, s_ycolp[:, li, c : c + 1],
                            usb[:, c, :], OP.mult, OP.add,
                        )
                        if not last:
                            if c % RELU_SPLIT == 0:
                                nc.vector.tensor_scalar_max(hn[:, c, :], usb[:, c, :], 0.0)
                            else:
                                nc.gpsimd.tensor_scalar_max(hn[:, c, :], usb[:, c, :], 0.0)
                        else:
                            ot = outp.tile([128, NB], BF, tag="ot")
                            nc.vector.tensor_tensor(ot, usb[:, c, :], mbs, OP.mult)
                            if c % RELU_SPLIT == 0:
                                nc.vector.tensor_scalar_max(ot, ot, 0.0)
                            else:
                                nc.gpsimd.tensor_scalar_max(ot, ot, 0.0)
                            nc.sync.dma_start(out=out_v[:, c, ncol], in_=ot)
                    if not last:
                        u = hn

    nc.compile()
    return nc


_NC_CACHE = {}


def _get_program(ntiles=NTILES):
    if ntiles not in _NC_CACHE:
        _NC_CACHE[ntiles] = build_program(ntiles)
    return _NC_CACHE[ntiles]


def make_in_maps(inputs, ntiles=NTILES):
    """Per-core input maps for a program with `ntiles` tiles."""
    hk = np.asarray(inputs["hk"], dtype=np.float32)
    w = np.asarray(inputs["w"], dtype=np.float32)
    b_list = [np.asarray(inputs[f"b{i}"], dtype=np.float32) for i in range(NUM_LAYERS)]
    prep = host_prep(w, b_list)
    hkb = hk.astype(BF_NP)
    rows_per = ntiles * NB
    in_maps = []
    for c in range(NCORES):
        m = dict(prep)
        m["hkr"] = hkb[c * ROWS_PER_CORE : c * ROWS_PER_CORE + rows_per]
        in_maps.append(m)
    return in_maps


def kernel(**inputs):
    nc = _get_program()
    in_maps = make_in_maps(inputs)
    res = bass_utils.run_bass_kernel_spmd(nc, in_maps, list(range(NCORES)))
    outs = [
        np.asarray(res.results[c]["outT"]).T.astype(np.float32) for c in range(NCORES)
    ]
    return np.ascontiguousarray(np.concatenate(outs, axis=0))


# revision 3
# speedup vs baseline: 2.2672x; 2.2672x over previous
"""Trainium2 Bass kernel for nn_HDCNN (4-layer hyperbolic dilated CNN), v2.

Data-parallel over 8 NeuronCores (4096 rows each). Feature-transposed layout:
activations live as [feature, batch] bf16 tiles; the 64-tap full convolution is
2 dense bf16 128x128 matmuls per output 128-chunk (W_lo/W_hi band matrices).

Math (same reduction as v1, validated against the reference): the Poincare
projection always triggers, so each layer is
    h' = relu(u + delta * y),  u = conv(h), delta = (CY/M) * n / cx,
    n = ||u||, cx = 1 + y2 + 2*M*<u,y>/n,
with the final layer scaled by m = M*cx/(n*den). Per-sample stats (s = n^2 via
squares, d = <u,y> via the precomputed cross-correlation beta on h) are reduced
with column-packed M=2 matmuls on 4 concurrent PE column strips.

Precision: bf16 inputs/weights/activations with fp32 PSUM accumulation.
Host-side numpy simulation of this exact pipeline measures l2 ~ 2.6e-3 vs the
fp32 reference (tolerance 2e-2).
"""
import os
import sys

for _p in ("/opt/trn_rl_repo", "/root/.axon_site/_ro/trn_rl_repo"):
    if _p not in sys.path and os.path.isdir(_p):
        sys.path.append(_p)

import numpy as np
import ml_dtypes
import concourse.bacc as bacc
import concourse.mybir as mybir
import concourse.tile as tile
from concourse import bass_utils

F32 = mybir.dt.float32
BF = mybir.dt.bfloat16
OP = mybir.AluOpType
AX = mybir.AxisListType
BF_NP = ml_dtypes.bfloat16

NCORES = 8
BATCH = 32768
INSIZE = 1024
FLEN = 64
NUM_LAYERS = 4
ROWS_PER_CORE = BATCH // NCORES          # 4096
NB = 512                                  # batch columns per tile
NTILES = ROWS_PER_CORE // NB              # 8
MAXNORM = 1.0 - 4e-3
COEF_Y = 1.0 - MAXNORM * MAXNORM

LIN = [INSIZE + FLEN * i for i in range(NUM_LAYERS)]          # 1024 1088 1152 1216
LOUT = [l + FLEN for l in LIN]                                # 1088 1152 1216 1280
NIN_C = [(l + 127) // 128 for l in LIN]                       # 8 9 9 10
NOUT_C = [(l + 127) // 128 for l in LOUT]                     # 9 9 10 10
NINMAX = max(NIN_C)
NOUTMAX = max(NOUT_C)

# Fraction denominator for relu placement: c % RELU_SPLIT == 0 goes to DVE,
# the rest to GPSIMD.
RELU_SPLIT = int(os.environ.get("HDCNN_RELU_SPLIT", "3"))


def host_prep(w, b_list):
    """Replicated parameter layouts (band matrices + correlations), bf16."""
    prep = {}
    wlo = np.zeros((NUM_LAYERS, 128, 128), np.float32)
    whi = np.zeros((NUM_LAYERS, 128, 128), np.float32)
    k = np.arange(128)[:, None]
    m = np.arange(128)[None, :]
    for i in range(NUM_LAYERS):
        dif = m - k
        sel = (dif >= 0) & (dif < FLEN)
        whi[i][sel] = w[i][dif[sel]]
        dif2 = m + 128 - k
        sel2 = (dif2 >= 0) & (dif2 < FLEN)
        wlo[i][sel2] = w[i][dif2[sel2]]
    prep["wlo"] = wlo.astype(BF_NP)
    prep["whi"] = whi.astype(BF_NP)

    sones2 = np.zeros((128, 2), np.float32)
    sones2[:, 0] = 1.0
    prep["sones2"] = sones2.astype(BF_NP)

    beta2 = np.zeros((NUM_LAYERS, 128, NINMAX, 2), np.float32)
    ycolp = np.zeros((NUM_LAYERS, 128, NOUTMAX), np.float32)
    y2c = np.zeros((128, NUM_LAYERS, 2), np.float32)
    for i in range(NUM_LAYERS):
        b64 = b_list[i].astype(np.float64)
        bt = np.correlate(b64, w[i].astype(np.float64), mode="valid")[: LIN[i]]
        bpad = np.zeros(NIN_C[i] * 128)
        bpad[: LIN[i]] = bt
        beta2[i, :, : NIN_C[i], 1] = bpad.reshape(NIN_C[i], 128).T
        ypad = np.zeros(NOUT_C[i] * 128)
        ypad[: LOUT[i]] = b64 * (COEF_Y / MAXNORM)
        ycolp[i, :, : NOUT_C[i]] = ypad.reshape(NOUT_C[i], 128).T
        y2 = np.float32(np.sum(b_list[i].astype(np.float32) ** 2, dtype=np.float32))
        y2c[:, i, 0] = np.float32(1.0) + y2                                # cx const
        y2c[:, i, 1] = np.float32(1.0) + np.float32(MAXNORM * MAXNORM) * y2  # den const
    prep["beta2"] = beta2.astype(BF_NP)
    prep["ycolp"] = ycolp.astype(BF_NP)
    prep["y2c"] = y2c

    # Broadcast selectors: ebc[b] is lhsT [8, 128]; row b set to 1 (b<4, delta
    # blocks) or MAXNORM (b>=4, m blocks), everything else 0.
    ebc = np.zeros((8, 8, 128), np.float32)
    for b in range(8):
        ebc[b, b, :] = 1.0 if b < 4 else MAXNORM
    prep["ebc"] = ebc.astype(BF_NP)

    prep["idbf"] = np.eye(128, dtype=BF_NP)
    prep["idf"] = np.eye(128, dtype=np.float32)
    return prep


def build_program(ntiles=NTILES, reps=1):
    nc = bacc.Bacc("TRN2", target_bir_lowering=False, debug=False)
    ncols = ntiles * NB

    hkT = nc.dram_tensor("hkT", [INSIZE, ncols], BF, kind="ExternalInput")
    hk_v = hkT.rearrange("(c p) n -> p c n", p=128)
    d_wlo = nc.dram_tensor("wlo", [NUM_LAYERS, 128, 128], BF, kind="ExternalInput")
    d_whi = nc.dram_tensor("whi", [NUM_LAYERS, 128, 128], BF, kind="ExternalInput")
    d_sones2 = nc.dram_tensor("sones2", [128, 2], BF, kind="ExternalInput")
    d_beta2 = nc.dram_tensor("beta2", [NUM_LAYERS, 128, NINMAX, 2], BF, kind="ExternalInput")
    d_ycolp = nc.dram_tensor("ycolp", [NUM_LAYERS, 128, NOUTMAX], BF, kind="ExternalInput")
    d_y2c = nc.dram_tensor("y2c", [128, NUM_LAYERS, 2], F32, kind="ExternalInput")
    d_ebc = nc.dram_tensor("ebc", [8, 8, 128], BF, kind="ExternalInput")
    d_idbf = nc.dram_tensor("idbf", [128, 128], BF, kind="ExternalInput")
    d_idf = nc.dram_tensor("idf", [128, 128], F32, kind="ExternalInput")
    outT = nc.dram_tensor("outT", [LOUT[-1], ncols], BF, kind="ExternalOutput")
    out_v = outT.rearrange("(c p) n -> p c n", p=128)

    with tile.TileContext(nc) as tc:
        with (
            tc.tile_pool(name="singles", bufs=1) as singles,
            tc.tile_pool(name="hkp", bufs=2) as hkp,
            tc.tile_pool(name="usbp", bufs=2) as usbp,
            tc.tile_pool(name="hp", bufs=2) as hp,
            tc.tile_pool(name="sqp", bufs=4) as sqp,
            tc.tile_pool(name="sdp", bufs=2) as sdp,
            tc.tile_pool(name="scp", bufs=2) as scp,
            tc.tile_pool(name="rowp", bufs=2) as rowp,
            tc.tile_pool(name="dbp", bufs=2) as dbp,
            tc.tile_pool(name="outp", bufs=4) as outp,
            tc.tile_pool(name="cvps", bufs=3, space="PSUM") as cvps,
            tc.tile_pool(name="stps", bufs=1, space="PSUM") as stps,
            tc.tile_pool(name="rwps", bufs=2, space="PSUM") as rwps,
            tc.tile_pool(name="bcps", bufs=1, space="PSUM") as bcps,
        ):
            s_wlo = singles.tile([128, NUM_LAYERS, 128], BF, tag="wlo")
            s_whi = singles.tile([128, NUM_LAYERS, 128], BF, tag="whi")
            s_sones2 = singles.tile([128, 2], BF, tag="sones2")
            s_beta2 = singles.tile([128, NUM_LAYERS, NINMAX, 2], BF, tag="beta2")
            s_ycolp = singles.tile([128, NUM_LAYERS, NOUTMAX], BF, tag="ycolp")
            s_y2c = singles.tile([128, NUM_LAYERS, 2], F32, tag="y2c")
            s_ebc = singles.tile([8, 8, 128], BF, tag="ebc")
            s_idbf = singles.tile([128, 128], BF, tag="idbf")
            s_idf = singles.tile([128, 128], F32, tag="idf")
            nc.sync.dma_start(out=s_wlo, in_=d_wlo.rearrange("l p m -> p l m"))
            nc.sync.dma_start(out=s_whi, in_=d_whi.rearrange("l p m -> p l m"))
            nc.sync.dma_start(out=s_sones2, in_=d_sones2[:])
            nc.sync.dma_start(out=s_beta2, in_=d_beta2.rearrange("l p c t -> p l c t"))
            nc.sync.dma_start(out=s_ycolp, in_=d_ycolp.rearrange("l p c -> p l c"))
            nc.sync.dma_start(out=s_y2c, in_=d_y2c[:])
            nc.sync.dma_start(out=s_ebc, in_=d_ebc.rearrange("b r m -> r b m"))
            nc.sync.dma_start(out=s_idbf, in_=d_idbf[:])
            nc.sync.dma_start(out=s_idf, in_=d_idf[:])

            for jj in range(ntiles * reps):
                j = jj % ntiles
                ncol = slice(j * NB, (j + 1) * NB)

                hkt = hkp.tile([128, NIN_C[0], NB], BF, tag="hk")
                nc.sync.dma_start(out=hkt, in_=hk_v[:, :, ncol])
                u = hkt

                for li in range(NUM_LAYERS):
                    nin_c, nout_c = NIN_C[li], NOUT_C[li]
                    last = li == NUM_LAYERS - 1
                    strip_last_s = {
                        sj: max(c for c in range(nout_c) if c % 4 == sj)
                        for sj in range(4)
                    }

                    # --- stats tile; d-part first (only needs layer input) ---
                    ST = stps.tile([128, NB], F32, tag="st")
                    for c in range(nin_c):
                        sj = c % 4
                        nc.tensor.matmul(
                            ST[32 * sj : 32 * sj + 2, :],
                            s_beta2[:, li, c, :],
                            u[:, c, :],
                            start=(c < 4),
                            stop=False,
                            tile_position=(0, 32 * sj),
                        )

                    # --- conv chunks: 2 bf16 matmuls each; evac + square ---
                    usb = usbp.tile([128, nout_c, NB], BF, tag=f"u{li}")
                    for c in range(nout_c):
                        pcv = cvps.tile([128, NB], F32, tag="cv")
                        ops = []
                        if 1 <= c <= nin_c:
                            ops.append((s_wlo[:, li, :], c - 1))
                        if c < nin_c:
                            ops.append((s_whi[:, li, :], c))
                        for oi, (lhs, uc) in enumerate(ops):
                            nc.tensor.matmul(
                                pcv, lhs, u[:, uc, :],
                                start=(oi == 0), stop=(oi == len(ops) - 1),
                            )
                        nc.scalar.copy(usb[:, c, :], pcv)
                        sq = sqp.tile([128, NB], BF, tag="sq")
                        nc.vector.tensor_tensor(sq, usb[:, c, :], usb[:, c, :], OP.mult)
                        sj = c % 4
                        nc.tensor.matmul(
                            ST[32 * sj : 32 * sj + 2, :],
                            s_sones2,
                            sq,
                            start=False,
                            stop=(c == strip_last_s[sj]),
                            tile_position=(0, 32 * sj),
                        )

                    # --- stats -> per-sample scalars ([128, 4] layout) ---
                    sd = sdp.tile([128, NB], F32, tag="sd")
                    nc.scalar.copy(sd, ST)
                    tp4 = stps.tile([128, 4, 128], F32, tag="tp4")
                    for b in range(4):
                        nc.tensor.transpose(
                            tp4[:, b, :], sd[:, 128 * b : 128 * (b + 1)], s_idf
                        )
                    sc = scp.tile([128, 28], F32, tag="sc")
                    S4, D4 = sc[:, 0:4], sc[:, 4:8]
                    nS, rn = sc[:, 8:12], sc[:, 12:16]
                    t0, cx, rcx = sc[:, 16:20], sc[:, 20:24], sc[:, 24:28]
                    nc.vector.tensor_reduce(S4, tp4[:, :, 0:128:32], AX.X, OP.add)
                    nc.vector.tensor_reduce(D4, tp4[:, :, 1:128:32], AX.X, OP.add)
                    nc.scalar.sqrt(nS, S4)
                    nc.vector.reciprocal(rn, nS)
                    nc.vector.tensor_tensor(t0, D4, rn, OP.mult)
                    nc.vector.tensor_scalar(
                        cx, t0, 2.0 * MAXNORM, s_y2c[:, li, 0:1], OP.mult, OP.add
                    )
                    nc.vector.reciprocal(rcx, cx)
                    dm = scp.tile([128, 8], F32, tag="dm")
                    nc.vector.tensor_tensor(dm[:, 0:4], nS, rcx, OP.mult)  # delta
                    if not last:
                        # rows 4..7 of the transposed block feed the (unused)
                        # m-broadcast path; zero them so stale SBUF/psum
                        # garbage (possibly NaN) never reaches a matmul.
                        nc.vector.memset(dm[:, 4:8], 0.0)
                    if last:
                        sc2 = scp.tile([128, 12], F32, tag="sc2")
                        den, rden, m1 = sc2[:, 0:4], sc2[:, 4:8], sc2[:, 8:12]
                        nc.vector.tensor_scalar(
                            den, t0, 2.0 * MAXNORM, s_y2c[:, li, 1:2], OP.mult, OP.add
                        )
                        nc.vector.reciprocal(rden, den)
                        nc.vector.tensor_tensor(m1, cx, rn, OP.mult)
                        nc.vector.tensor_tensor(dm[:, 4:8], m1, rden, OP.mult)  # m/M

                    # --- broadcast delta (and m) to [128, NB] bf16 tiles ---
                    rw = rwps.tile([8, 128], F32, tag="rw")
                    nc.tensor.transpose(rw, dm, s_idf)
                    rws = rowp.tile([8, 128], BF, tag="rws")
                    nc.scalar.copy(rws, rw)
                    dbc = bcps.tile([128, NB], F32, tag="bc")
                    for b in range(4):
                        nc.tensor.matmul(
                            dbc[:, 128 * b : 128 * (b + 1)],
                            s_ebc[:, b, :],
                            rws[:, :],
                            start=True, stop=True, tile_position=(0, 0),
                        )
                    dbs = dbp.tile([128, NB], BF, tag="dbs")
                    nc.scalar.copy(dbs, dbc)
                    if last:
                        mbc = bcps.tile([128, NB], F32, tag="bc")
                        for b in range(4):
                            nc.tensor.matmul(
                                mbc[:, 128 * b : 128 * (b + 1)],
                                s_ebc[:, 4 + b, :],
                                rws[:, :],
                                start=True, stop=True, tile_position=(0, 0),
                            )
                        mbs = dbp.tile([128, NB], BF, tag="mbs")
                        nc.scalar.copy(mbs, mbc)

                    # --- q update (in place), relu, output ---
                    if not last:
                        hn = hp.tile([128, nout_c, NB], BF, tag=f"h{li + 1}")
                    else:
                        hn = outp.tile([128, nout_c, NB], BF, tag="ot")
                    for c in range(nout_c):
                        nc.vector.scalar_tensor_tensor(
                            usb[:, c, :], dbs, s_ycolp[:, li, c : c + 1],
                            usb[:, c, :], OP.mult, OP.add,
                        )
                        if last:
                            nc.vector.tensor_tensor(
                                usb[:, c, :], usb[:, c, :], mbs, OP.mult
                            )
                        if c % RELU_SPLIT == 0:
                            nc.vector.tensor_scalar_max(hn[:, c, :], usb[:, c, :], 0.0)
                        else:
                            nc.gpsimd.tensor_scalar_max(hn[:, c, :], usb[:, c, :], 0.0)
                    if not last:
                        u = hn
                    else:
                        nc.sync.dma_start(out=out_v[:, :, ncol], in_=hn)

    nc.compile()
    return nc


_NC_CACHE = {}


def _get_program(ntiles=NTILES):
    if ntiles not in _NC_CACHE:
        _NC_CACHE[ntiles] = build_program(ntiles)
    return _NC_CACHE[ntiles]


def make_in_maps(inputs, ntiles=NTILES):
    """Per-core input maps for a program with `ntiles` tiles."""
    hk = np.asarray(inputs["hk"], dtype=np.float32)
    w = np.asarray(inputs["w"], dtype=np.float32)
    b_list = [np.asarray(inputs[f"b{i}"], dtype=np.float32) for i in range(NUM_LAYERS)]
    prep = host_prep(w, b_list)
    rows_per = ntiles * NB
    in_maps = []
    for c in range(NCORES):
        m = dict(prep)
        rows = hk[c * ROWS_PER_CORE : c * ROWS_PER_CORE + rows_per]
        m["hkT"] = np.ascontiguousarray(rows.T).astype(BF_NP)
        in_maps.append(m)
    return in_maps


def kernel(**inputs):
    nc = _get_program()
    in_maps = make_in_maps(inputs)
    res = bass_utils.run_bass_kernel_spmd(nc, in_maps, list(range(NCORES)))
    outs = [
        np.asarray(res.results[c]["outT"]).T.astype(np.float32) for c in range(NCORES)
    ]
    return np.ascontiguousarray(np.concatenate(outs, axis=0))
